# revision 9
# baseline (speedup 1.0000x reference)
"""Causal multi-head attention layer on 8 Trainium2 NeuronCores.

Sharding: tensor-parallel over heads (16 heads -> 2 per core).

Numerics / dataflow (per core, its 2 heads):
  - QKV projection: error-compensated fp8e4m3 DoubleRow matmuls:
      q*2^14 = x8@W8 + xr8@W8 + x8@Wr8
    with x8=fp8(xT*16), xr8=fp8(xT*16-x8), W8=fp8(W*1024),
    Wr8=fp8(W*1024-W8).  All three terms share one PSUM scale (2^14);
    the DVE bias-add rescales by 2^-14 and adds b.  ~bf16 accuracy at
    0.75x the bf16 PE cost (DoubleRow streams 2 rows/cycle).
  - scores S^T[k,q] = K^T_chunk^T @ Q^T in bf16; two heads run
    concurrently via tile_position row groups into one 2-bank PSUM
    tile; ONE exp activation per 512-piece covers both heads (3D AP).
  - att@V: off-diagonal k-blocks (k < qa-512) as fp8 DoubleRow pairs
    (4x fewer PE cycles); near-diagonal window stays bf16.  Ones
    column folded into the V stationary gives the softmax denominator.
  - output projection bf16; per-core partials DMA'd to DRAM in bf16;
    host sums over cores and adds b_out.
"""
import numpy as np
import ml_dtypes

import concourse.bacc as bacc
import concourse.bass as bass
import concourse.mybir as mybir
import concourse.tile as tile
from concourse import bass_utils

B, S, E, H = 4, 2048, 1024, 16
D = E // H            # 64
TOK = B * S           # 8192
KC = E // 128         # 8 contraction chunks of 128
G = KC // 2           # 4 DoubleRow groups of 256
TB = 512              # psum token block
QB = 1024             # attention q block
NQB = S // QB         # 2 q blocks per batch
USCL = float(2.0 ** -14)  # undo x*16 @ W*1024 scaling
FP8_ATT = False   # fp8 DoubleRow for off-diagonal att@V
FP8_QKV = True   # fp8 DoubleRow compensated QKV projection

f32 = mybir.dt.float32
f8 = mybir.dt.float8e4
bf16 = mybir.dt.bfloat16
FT = mybir.ActivationFunctionType
DR = mybir.MatmulPerfMode.DoubleRow
MULT = mybir.AluOpType.mult
ADD = mybir.AluOpType.add


def splits(lo, hi, step=512):
    """Split [lo, hi) into pieces aligned to `step` boundaries."""
    out = []
    p = lo
    while p < hi:
        q = min((p // step + 1) * step, hi)
        out.append((p, q))
        p = q
    return out


def ap3(base, step, n2, inner):
    """3D AP: [partition, [step, n2], [1, inner]] at base's offset."""
    return bass.AP(base.tensor, base.offset, [base.ap[0], [step, n2], [1, inner]])


def build(repeats: int = 1):
    nc = bacc.Bacc("TRN2", target_bir_lowering=False, debug=False, num_devices=8)
    xT8 = nc.dram_tensor("xT8", [E, TOK], f8, kind="ExternalInput")
    xrT8 = nc.dram_tensor("xrT8", [E, TOK], f8, kind="ExternalInput")
    wdr = {}
    for p in ("q", "k", "v"):
        wdr[p] = nc.dram_tensor(f"w8{p}", [E, 128], f8, kind="ExternalInput")
        wdr[p + "r"] = nc.dram_tensor(f"wr8{p}", [E, 128], f8, kind="ExternalInput")
    wo = nc.dram_tensor("wo", [128, E], bf16, kind="ExternalInput")
    bq = nc.dram_tensor("bq", [128, 1], f32, kind="ExternalInput")
    bk = nc.dram_tensor("bk", [128, 1], f32, kind="ExternalInput")
    bv = nc.dram_tensor("bv", [128, 1], f32, kind="ExternalInput")
    tri = nc.dram_tensor("tri", [128, 128], bf16, kind="ExternalInput")
    idd = nc.dram_tensor("idd", [128, 128], bf16, kind="ExternalInput")
    outp = nc.dram_tensor("outp", [E, TOK], bf16, kind="ExternalOutput")

    with tile.TileContext(nc) as tc:
        with (
            tc.tile_pool(name="wp", bufs=1) as wp,
            tc.tile_pool(name="xp", bufs=2) as xp,
            tc.tile_pool(name="qk", bufs=2) as qk,
            tc.tile_pool(name="vn", bufs=1) as vnp,
            tc.tile_pool(name="at", bufs=2) as atp,
            tc.tile_pool(name="ao", bufs=2) as aop,
            tc.tile_pool(name="ms", bufs=1) as ms,
            tc.tile_pool(name="op", bufs=3) as op,
            tc.tile_pool(name="psA", bufs=2, space="PSUM") as psA,
            tc.tile_pool(name="psS", bufs=2, space="PSUM") as psS,
            tc.tile_pool(name="psO", bufs=2, space="PSUM") as psO,
        ):
            # --- weights / constants (loaded once) ---
            wsb = {}
            for p in ("q", "k", "v"):
                for r in ("", "r"):
                    w = wp.tile([128, KC * 128], f8, tag=f"w8{p}{r}",
                                name=f"w8{p}{r}")
                    nc.sync.dma_start(
                        w[:].rearrange("p (c m) -> p c m", c=KC),
                        wdr[p + r].ap().rearrange("(c p) m -> p c m", p=128),
                    )
                    wsb[p + r] = w
            # first QKV round's x so PE can start as soon as weights land
            xts00 = []
            for src in (xT8, xrT8):
                x1 = xp.tile([128, KC * QB], f8, tag=f"x{len(xts00)}",
                             name=f"xt_pre0_{len(xts00)}")
                nc.sync.dma_start(
                    x1[:].rearrange("p (c m) -> p c m", c=KC),
                    src.ap()[:, 0:QB].rearrange("(c p) m -> p c m", p=128),
                )
                xts00.append(x1)
            wo_sb = wp.tile([128, E], bf16)
            nc.sync.dma_start(wo_sb[:], wo.ap())
            bsb = {}
            for p, bd in (("q", bq), ("k", bk), ("v", bv)):
                t = wp.tile([128, 1], f32, tag=f"b{p}", name=f"b{p}")
                nc.sync.dma_start(t[:], bd.ap())
                bsb[p] = t
            tri_sb = wp.tile([128, 128], bf16)
            nc.sync.dma_start(tri_sb[:], tri.ap())
            id_sb = wp.tile([128, 128], bf16)
            nc.sync.dma_start(id_sb[:], idd.ap())
            # preload ACT exp table during the prologue
            warm = wp.tile([1, 1], f32)
            nc.vector.memset(warm[:], 0.0)
            nc.scalar.activation(warm[:], warm[:], FT.Exp, scale=1.0)
            # persistent V tiles: bf16 (diag window) + fp8 pairs (DoubleRow)
            vns = []
            for i in range(S // 128):
                vn = vnp.tile([128, 130], bf16, tag=f"vn{i}", name=f"vn{i}")
                nc.vector.memset(vn[:, 64:65], 1.0)
                nc.vector.memset(vn[:, 129:130], 1.0)
                vns.append(vn)
            vn8s = []
            for j in range(S // 256):
                v8 = vnp.tile([128, 320], f8, tag=f"v8{j}", name=f"v8{j}")
                for j2 in range(2):
                    nc.vector.memset(v8[:, j2 * 160 + 64:j2 * 160 + 65], 1.0)
                    nc.vector.memset(v8[:, j2 * 160 + 144:j2 * 160 + 145], 1.0)
                vn8s.append(v8)

            def alloc_qkv(b):
                return (
                    qk.tile([128, S], bf16, tag="qT", name=f"qT{b}"),
                    qk.tile([128, S], bf16, tag="kT", name=f"kT{b}"),
                    qk.tile([128, S], bf16, tag="vT", name=f"vT{b}"),
                )

            def qkv_dma(b, t2, rep):
                tok0 = b * S + t2 * QB
                xts = []
                for i, src in enumerate((xT8, xrT8)):
                    x1 = xp.tile([128, KC * QB], f8, tag=f"x{i}",
                                 name=f"xt{rep}_{b}_{t2}_{i}")
                    nc.sync.dma_start(
                        x1[:].rearrange("p (c m) -> p c m", c=KC),
                        src.ap()[:, tok0:tok0 + QB].rearrange(
                            "(c p) m -> p c m", p=128),
                    )
                    xts.append(x1)
                return xts

            def qkv_gi(b, t2, tiles, xts, gi, rep):
                """One projection for one 1024-token round: 24 fp8 DR
                matmuls (3 compensation terms x 4 groups x 2 blocks)."""
                p = "qkv"[gi]
                dst = tiles[gi]
                x8t, xr8t = xts
                ps = [psA.tile([128, TB], f32, tag="mm512",
                               name=f"psqkv{rep}_{b}_{t2}_{gi}_{tb}")
                      for tb in range(2)]
                for g in range(G):
                    for (wt, xt) in ((wsb[p], x8t), (wsb[p], xr8t),
                                     (wsb[p + "r"], x8t)):
                        for tb in range(2):
                            nc.tensor.matmul(
                                ps[tb][:],
                                ap3(wt[:, 2 * g * 128:2 * g * 128 + 1],
                                    128, 2, 128),
                                ap3(xt[:, 2 * g * QB + tb * TB:
                                       2 * g * QB + tb * TB + 1],
                                    QB, 2, TB),
                                start=(g == 0 and xt is x8t and wt is wsb[p]),
                                stop=(g == G - 1 and wt is wsb[p + "r"]),
                                perf_mode=DR,
                            )
                for tb in range(2):
                    nc.vector.tensor_scalar(
                        dst[:, t2 * QB + tb * TB:t2 * QB + (tb + 1) * TB],
                        ps[tb][:], USCL, bsb[p][:], op0=MULT, op1=ADD,
                    )

            def vnat(b, tiles, rep, lo=0, hi=S // 128):
                vT = tiles[2]
                for i in range(lo, hi):
                    pst = psA.tile([128, 128], bf16, tag="mm512",
                                   name=f"pst{rep}_{b}_{i}")
                    nc.tensor.transpose(
                        pst[:], vT[:, i * 128:(i + 1) * 128], id_sb[:]
                    )
                    src3 = ap3(pst[:, 0:1], 64, 2, 64)
                    nc.vector.tensor_copy(
                        ap3(vns[i][:, 0:1], 65, 2, 64), src3)
                    nc.vector.tensor_copy(
                        ap3(vn8s[i // 2][:, (i % 2) * 160:(i % 2) * 160 + 1],
                            80, 2, 64), src3)

            def alloc_atts(b, qb, rep):
                nm = f"{rep}_{b}_{qb}"
                f8a = f8 if FP8_ATT else bf16
                return {
                    "a8A": atp.tile([128, 8 * QB], f8a, tag="a8A", name=f"a8A{nm}"),
                    "a8B": atp.tile([128, 8 * 512], f8a, tag="a8B", name=f"a8B{nm}"),
                    "aw": atp.tile([128, 8 * 512], bf16, tag="aw", name=f"aw{nm}"),
                    "abA": atp.tile([128, 8 * QB], bf16, tag="abA", name=f"abA{nm}"),
                    "abB": atp.tile([128, 8 * 512], bf16, tag="abB", name=f"abB{nm}"),
                }

            def scores(b, qb, tiles, atts, rep, fill=()):
                qT, kT, vT = tiles
                q0 = qb * QB
                nd = q0 // 128
                nkc = (q0 + QB) // 128
                fill = list(fill)
                nf = len(fill)
                fired = 0
                for kc in range(nkc):
                    kst = kc * 128
                    r0 = max(0, kst - q0)
                    pcs = splits(r0, QB)
                    pss = [
                        psS.tile([128, 1024], f32, tag="s",
                                 name=f"pss{rep}_{b}_{qb}_{kc}_{p0}")
                        for (p0, p1) in pcs
                    ]
                    for h in range(2):
                        hs = slice(h * 64, (h + 1) * 64)
                        for (p0, p1), ps in zip(pcs, pss):
                            nc.tensor.matmul(
                                ps[:, h * 512:h * 512 + (p1 - p0)],
                                kT[hs, kst:kst + 128],
                                qT[hs, q0 + p0:q0 + p1],
                                start=True, stop=True,
                                tile_position=(h * 64, 0),
                            )
                    for (p0, p1), ps in zip(pcs, pss):
                        n = p1 - p0
                        qa = q0 + (512 if p0 >= 512 else 0)
                        if kst < qa - 512:
                            if kc < nd - 4:
                                base = atts["a8A"][:, 2 * kc * QB + p0:
                                                   2 * kc * QB + p0 + 1]
                                dst = ap3(base, QB, 2, n)
                            else:
                                e = kc - (nd - 4)
                                base = atts["a8B"][:, 2 * e * 512 + p0 - 512:
                                                   2 * e * 512 + p0 - 511]
                                dst = ap3(base, 512, 2, n)
                        elif kc < nd:
                            e = kc - (nd - 4)
                            base = atts["aw"][:, 2 * e * 512 + p0:
                                              2 * e * 512 + p0 + 1]
                            dst = ap3(base, 512, 2, n)
                        elif kc < nd + 4:
                            e = kc - nd
                            base = atts["abA"][:, 2 * e * QB + p0:
                                               2 * e * QB + p0 + 1]
                            dst = ap3(base, QB, 2, n)
                        else:
                            e = kc - nd - 4
                            base = atts["abB"][:, 2 * e * 512 + p0 - 512:
                                               2 * e * 512 + p0 - 511]
                            dst = ap3(base, 512, 2, n)
                        nc.scalar.activation(
                            dst, ap3(ps[:, 0:1], 512, 2, n),
                            FT.Exp, scale=0.125,
                        )
                    if kst >= q0:
                        # causal trim on the diagonal 128x128 block
                        for h in range(2):
                            if kc < nd + 4:
                                e = kc - nd
                                blk = atts["abA"][:, (2 * e + h) * QB + r0:
                                                  (2 * e + h) * QB + r0 + 128]
                            else:
                                e = kc - nd - 4
                                blk = atts["abB"][:, (2 * e + h) * 512 + r0 - 512:
                                                  (2 * e + h) * 512 + r0 - 384]
                            nc.vector.tensor_tensor(
                                blk, blk, tri_sb[:], op=MULT)
                    want = (kc + 1) * nf // nkc
                    while fired < want:
                        fill[fired]()
                        fired += 1
                while fired < nf:
                    fill[fired]()
                    fired += 1

            def attv(b, qb, qbb, h, atts, aos, rep):
                q0 = qb * QB
                nd = q0 // 128
                qa = q0 + qbb * 512
                ps_o = psO.tile([65, 512], f32, tag="o",
                                name=f"pso{rep}_{b}_{qb}_{qbb}_{h}")
                started = False
                # fp8 DoubleRow pairs (k-blocks fully below qa-512)
                nf8 = max(nd - 4 + qbb * 4, 0)
                if FP8_ATT:
                    for j in range(nf8 // 2):
                        if 2 * j + 1 < nd - 4:
                            base = atts["a8A"][:, (4 * j + h) * QB + qbb * 512:
                                               (4 * j + h) * QB + qbb * 512 + 1]
                            rhs = ap3(base, 2 * QB, 2, 512)
                        else:
                            e = 2 * j - (nd - 4)
                            base = atts["a8B"][:, (2 * e + h) * 512:
                                               (2 * e + h) * 512 + 1]
                            rhs = ap3(base, 2 * 512, 2, 512)
                        nc.tensor.matmul(
                            ps_o[:, 0:512],
                            ap3(vn8s[j][:, h * 80:h * 80 + 1], 160, 2, 65),
                            rhs, start=(not started), stop=False, perf_mode=DR,
                        )
                        started = True
                else:
                    for kc in range(nf8):
                        if kc < nd - 4:
                            mov = atts["a8A"][:, (2 * kc + h) * QB + qbb * 512:
                                              (2 * kc + h) * QB + qbb * 512 + 512]
                        else:
                            e = kc - (nd - 4)
                            mov = atts["a8B"][:, (2 * e + h) * 512:
                                              (2 * e + h + 1) * 512]
                        nc.tensor.matmul(
                            ps_o[:, 0:512],
                            vns[kc][:, h * 65:(h + 1) * 65],
                            mov, start=(not started), stop=False,
                        )
                        started = True
                # bf16 full blocks in the near-diagonal window
                w0 = nd - 4 + qbb * 4
                for i in range(4):
                    kc = w0 + i
                    if kc < 0:
                        continue
                    if qbb == 0:
                        mov = atts["aw"][:, (2 * i + h) * 512:
                                         (2 * i + h + 1) * 512]
                    else:
                        mov = atts["abA"][:, (2 * i + h) * QB + 512:
                                          (2 * i + h) * QB + QB]
                    nc.tensor.matmul(
                        ps_o[:, 0:512],
                        vns[kc][:, h * 65:(h + 1) * 65],
                        mov, start=(not started), stop=False,
                    )
                    started = True
                # diagonal 4 blocks (bf16, trimmed)
                for i in range(4):
                    kc = nd + qbb * 4 + i
                    kst = kc * 128
                    lo = kst - qa
                    if qbb == 0:
                        e = i
                        mov = atts["abA"][:, (2 * e + h) * QB + lo:
                                          (2 * e + h) * QB + 512]
                    else:
                        e = i
                        mov = atts["abB"][:, (2 * e + h) * 512 + lo:
                                          (2 * e + h + 1) * 512]
                    nc.tensor.matmul(
                        ps_o[:, lo:512],
                        vns[kc][:, h * 65:(h + 1) * 65],
                        mov, start=(not started), stop=(i == 3),
                    )
                    started = True
                rec = ms.tile([1, 512], f32, tag=f"rec{h}",
                              name=f"rec{rep}_{b}_{qb}_{qbb}_{h}")
                nc.vector.reciprocal(rec[:], ps_o[64:65, :])
                bc = ms.tile([64, 512], f32, tag=f"bc{h}",
                             name=f"bc{rep}_{b}_{qb}_{qbb}_{h}")
                nc.gpsimd.partition_broadcast(bc[:], rec[:])
                nc.vector.tensor_tensor(
                    aos[h * 64:(h + 1) * 64, qa:qa + 512],
                    ps_o[0:64, :], bc[:], op=MULT,
                )

            def outproj_ec(b, half, ec, aos, rep):
                po = op.tile([128, QB], bf16, tag="po",
                             name=f"po{rep}_{b}_{ec}_{half}")
                for tt in range(2):
                    ps_p = psA.tile([128, TB], f32, tag="mm512",
                                    name=f"psp{rep}_{b}_{ec}_{half}_{tt}")
                    nc.tensor.matmul(
                        ps_p[:],
                        wo_sb[:, ec * 128:(ec + 1) * 128],
                        aos[:, half * QB + tt * TB:half * QB + (tt + 1) * TB],
                        start=True, stop=True,
                    )
                    # GPSIMD cannot read PSUM; split copies DVE/ACT
                    if (ec * 2 + tt) % 4 == 3:
                        nc.scalar.copy(po[:, tt * TB:(tt + 1) * TB], ps_p[:])
                    else:
                        nc.vector.tensor_copy(
                            po[:, tt * TB:(tt + 1) * TB], ps_p[:])
                nc.sync.dma_start(
                    outp.ap()[ec * 128:(ec + 1) * 128,
                              b * S + half * QB:b * S + (half + 1) * QB],
                    po[:],
                )

            for rep in range(repeats):
                # prologue: batch-0 tokens [0,1024) serially; the rest
                # becomes fill work inside the first scores loop
                tiles = alloc_qkv(0)
                xts = xts00 if rep == 0 else qkv_dma(0, 0, rep)
                for gi in range(3):
                    qkv_gi(0, 0, tiles, xts, gi, rep)
                vnat(0, tiles, rep, 0, 8)
                pro_fill = []
                xts1 = qkv_dma(0, 1, rep)
                for gi in range(3):
                    pro_fill.append(
                        (lambda gi=gi, tl=tiles, x=xts1:
                         qkv_gi(0, 1, tl, x, gi, rep))
                    )
                pro_fill.append(
                    (lambda tl=tiles: vnat(0, tl, rep, 8, S // 128))
                )
                prev = None  # (b, aos) with half-1 outproj still pending
                for b in range(B):
                    nxt = b + 1 if b + 1 < B else None
                    tiles_next = alloc_qkv(nxt) if nxt is not None else None
                    aos = aop.tile([128, S], bf16, tag="ao",
                                   name=f"ao{rep}_{b}")
                    for qb in range(NQB):
                        atts = alloc_atts(b, qb, rep)
                        fill = []
                        if b == 0 and qb == 0:
                            fill.extend(pro_fill)
                        if nxt is not None:
                            xts = qkv_dma(nxt, qb, rep)
                            for gi in range(3):
                                fill.append(
                                    (lambda gi=gi, x=xts, t2=qb:
                                     qkv_gi(nxt, t2, tiles_next, x, gi, rep))
                                )
                        if qb == 0 and prev is not None:
                            pb, paos = prev
                            for ec in range(KC):
                                fill.append(
                                    (lambda ec=ec, pb=pb, paos=paos:
                                     outproj_ec(pb, 1, ec, paos, rep))
                                )
                            prev = None
                        if qb == 1:
                            for ec in range(KC):
                                fill.append(
                                    (lambda ec=ec: outproj_ec(b, 0, ec,
                                                              aos, rep))
                                )
                        scores(b, qb, tiles, atts, rep, fill)
                        for qbb in range(QB // 512):
                            for h in range(2):
                                attv(b, qb, qbb, h, atts, aos, rep)
                    if nxt is not None:
                        vnat(nxt, tiles_next, rep)
                    prev = (b, aos)
                    tiles = tiles_next
                pb, paos = prev
                for ec in range(KC):
                    outproj_ec(pb, 1, ec, paos, rep)
    nc.compile()
    return nc


_CACHE = {}


def _get_nc(repeats=1):
    if repeats not in _CACHE:
        _CACHE[repeats] = build(repeats)
    return _CACHE[repeats]


E4 = ml_dtypes.float8_e4m3


def make_in_maps(x, W_qkv, b_qkv, W_out, b_out):
    x = np.asarray(x, dtype=np.float32)
    W_qkv = np.asarray(W_qkv, dtype=np.float32)
    b_qkv = np.asarray(b_qkv, dtype=np.float32)
    W_out = np.asarray(W_out, dtype=np.float32)
    xT = np.ascontiguousarray(x.reshape(TOK, E).T)
    x16 = xT * np.float32(16.0)
    x8 = x16.astype(E4)
    xr8 = (x16 - x8.astype(np.float32)).astype(E4)
    trim = np.ascontiguousarray(
        np.triu(np.ones((128, 128), dtype=np.float32))
    ).astype(ml_dtypes.bfloat16)
    ident = np.eye(128, dtype=np.float32).astype(ml_dtypes.bfloat16)
    in_maps = []
    for c in range(8):
        cs = slice(c * 128, (c + 1) * 128)
        m = {"xT8": x8, "xrT8": xr8, "tri": trim, "idd": ident,
             "wo": np.ascontiguousarray(W_out[cs, :]).astype(ml_dtypes.bfloat16)}
        for p, off in (("q", 0), ("k", E), ("v", 2 * E)):
            w = W_qkv[:, off + c * 128:off + (c + 1) * 128] * np.float32(1024.0)
            w8 = np.ascontiguousarray(w).astype(E4)
            wr8 = (w - w8.astype(np.float32)).astype(E4)
            m[f"w8{p}"] = w8
            m[f"wr8{p}"] = np.ascontiguousarray(wr8)
            m[f"b{p}"] = np.ascontiguousarray(
                b_qkv[off + c * 128:off + (c + 1) * 128, None])
        in_maps.append(m)
    return in_maps


def gather(results, b_out):
    total = np.zeros((E, TOK), dtype=np.float64)
    for c in range(8):
        total += results[c]["outp"].astype(np.float64)
    out = total.T.astype(np.float32) + np.asarray(b_out, dtype=np.float32)
    return np.ascontiguousarray(out.reshape(B, S, E)).astype(np.float32)


def kernel(x, W_qkv, b_qkv, W_out, b_out):
    nc = _get_nc(1)
    in_maps = make_in_maps(x, W_qkv, b_qkv, W_out, b_out)
    res = bass_utils.run_bass_kernel_spmd(nc, in_maps, core_ids=list(range(8)))
    return gather(res.results, b_out)


# revision 12
# speedup vs baseline: 2.0803x; 2.0803x over previous
"""Causal multi-head attention layer on 8 Trainium2 NeuronCores.

Sharding: tensor-parallel over heads (16 heads -> 2 per core).

All-bf16 dataflow (per core, its 2 heads), f32 PSUM accumulation:
  qkv^T = W_slice^T @ x^T          (bf16 matmuls, x pre-transposed host-side)
  S^T[k,q] = K^T_chunk^T @ Q^T     (bf16; the two heads run concurrently via
                                    tile_position row groups into one 2-bank
                                    PSUM tile; ONE exp covers both heads via
                                    a 3D AP)
  att^T = exp(S^T/8)   (bf16, causal-trimmed, triangular mask on diagonal)
  out^T[dv,q] = (V|1)^T @ att^T    (bf16; ones column gives denominator)
  attout^T = out^T[0:64] * bcast(1/denom)
  partial^T[e,tok] = W_out_slice @ attout^T -> DRAM (bf16)
Host: sum bf16 partials over cores in f64, transpose, + b_out.

Scores use 512-wide q blocks so each (k-chunk, q-block) needs a single
PSUM piece and a single merged-head exp; att tiles are double-buffered
so exp(qb+1) overlaps att@V(qb).
"""
import numpy as np
import ml_dtypes

import concourse.bacc as bacc
import concourse.bass as bass
import concourse.mybir as mybir
import concourse.tile as tile
from concourse import bass_utils

B, S, E, H = 4, 2048, 1024, 16
D = E // H            # 64
TOK = B * S           # 8192
KC = E // 128         # 8 contraction chunks of 128
TB = 512              # token block (psum free size)
NB = S // TB          # 4 token blocks per batch
NQ = S // TB          # 4 scores q-blocks per batch

f32 = mybir.dt.float32
bf16 = mybir.dt.bfloat16
FT = mybir.ActivationFunctionType
MULT = mybir.AluOpType.mult

BF = ml_dtypes.bfloat16


def ap3(base, step, n2, inner):
    """3D AP: [partition, [step, n2], [1, inner]] at base's offset."""
    return bass.AP(base.tensor, base.offset, [base.ap[0], [step, n2], [1, inner]])


def build(repeats: int = 1):
    nc = bacc.Bacc("TRN2", target_bir_lowering=False, debug=False, num_devices=8)
    xT = nc.dram_tensor("xT", [E, TOK], bf16, kind="ExternalInput")
    wq = nc.dram_tensor("wq", [E, 128], bf16, kind="ExternalInput")
    wk = nc.dram_tensor("wk", [E, 128], bf16, kind="ExternalInput")
    wv = nc.dram_tensor("wv", [E, 128], bf16, kind="ExternalInput")
    wo = nc.dram_tensor("wo", [128, E], bf16, kind="ExternalInput")
    bq = nc.dram_tensor("bq", [128, 1], f32, kind="ExternalInput")
    bk = nc.dram_tensor("bk", [128, 1], f32, kind="ExternalInput")
    bv = nc.dram_tensor("bv", [128, 1], f32, kind="ExternalInput")
    tri = nc.dram_tensor("tri", [128, 128], bf16, kind="ExternalInput")
    idd = nc.dram_tensor("idd", [128, 128], bf16, kind="ExternalInput")
    outp = nc.dram_tensor("outp", [E, TOK], bf16, kind="ExternalOutput")

    with tile.TileContext(nc) as tc:
        with (
            tc.tile_pool(name="wp", bufs=1) as wp,
            tc.tile_pool(name="xp", bufs=2) as xp,
            tc.tile_pool(name="qk", bufs=2) as qk,
            tc.tile_pool(name="vn", bufs=1) as vnp,
            tc.tile_pool(name="at", bufs=2) as atp,
            tc.tile_pool(name="ao", bufs=2) as aop,
            tc.tile_pool(name="ms", bufs=1) as ms,
            tc.tile_pool(name="op", bufs=3) as op,
            tc.tile_pool(name="psA", bufs=2, space="PSUM") as psA,
            tc.tile_pool(name="psS", bufs=2, space="PSUM") as psS,
            tc.tile_pool(name="psO", bufs=2, space="PSUM") as psO,
        ):
            # --- weights / constants (loaded once) ---
            # first QKV block's x loads FIRST so the PE can start early
            xt00 = xp.tile([128, KC * TB], bf16, tag="x", name="xt_pre0")
            nc.sync.dma_start(
                xt00[:].rearrange("p (c m) -> p c m", c=KC),
                xT.ap()[:, 0:TB].rearrange("(c p) m -> p c m", p=128),
            )
            wsb = {}
            for p, wd in (("q", wq), ("k", wk), ("v", wv)):
                w = wp.tile([128, KC * 128], bf16, tag=f"w{p}", name=f"w{p}")
                nc.sync.dma_start(
                    w[:].rearrange("p (c m) -> p c m", c=KC),
                    wd.ap().rearrange("(c p) m -> p c m", p=128),
                )
                wsb[p] = w
            bsb = {}
            for p, bd in (("q", bq), ("k", bk), ("v", bv)):
                t = wp.tile([128, 1], f32, tag=f"b{p}", name=f"b{p}")
                nc.sync.dma_start(t[:], bd.ap())
                bsb[p] = t
            wo_sb = wp.tile([128, E], bf16)
            nc.sync.dma_start(wo_sb[:], wo.ap())
            tri_sb = wp.tile([128, 128], bf16)
            nc.sync.dma_start(tri_sb[:], tri.ap())
            id_sb = wp.tile([128, 128], bf16)
            nc.sync.dma_start(id_sb[:], idd.ap())
            # preload the ACT exp table set during the prologue
            warm = wp.tile([1, 1], f32)
            nc.vector.memset(warm[:], 0.0)
            nc.scalar.activation(warm[:], warm[:], FT.Exp, scale=1.0)
            # persistent V-natural tiles; ones columns written once
            vns = []
            for i in range(S // 128):
                vn = vnp.tile([128, 130], bf16, tag=f"vn{i}", name=f"vn{i}")
                nc.vector.memset(vn[:, 64:65], 1.0)
                nc.vector.memset(vn[:, 129:130], 1.0)
                vns.append(vn)

            def alloc_qkv(b):
                return (
                    qk.tile([128, S], bf16, tag="qT", name=f"qT{b}"),
                    qk.tile([128, S], bf16, tag="kT", name=f"kT{b}"),
                    qk.tile([128, S], bf16, tag="vT", name=f"vT{b}"),
                )

            def qkv_dma(b, t, rep):
                tok0 = b * S + t * TB
                x1 = xp.tile([128, KC * TB], bf16, tag="x",
                             name=f"xt{rep}_{b}_{t}")
                nc.sync.dma_start(
                    x1[:].rearrange("p (c m) -> p c m", c=KC),
                    xT.ap()[:, tok0:tok0 + TB].rearrange(
                        "(c p) m -> p c m", p=128),
                )
                return x1

            def qkv_group(b, t, tiles, xt, gi, rep):
                p = "qkv"[gi]
                dst = tiles[gi]
                ps = psA.tile([128, TB], f32, tag="mm512",
                              name=f"psqkv{rep}_{b}_{t}_{gi}")
                for kc in range(KC):
                    nc.tensor.matmul(
                        ps[:],
                        wsb[p][:, kc * 128:(kc + 1) * 128],
                        xt[:, kc * TB:(kc + 1) * TB],
                        start=(kc == 0), stop=(kc == KC - 1),
                    )
                nc.vector.tensor_scalar_add(
                    dst[:, t * TB:(t + 1) * TB], ps[:], bsb[p][:]
                )

            def vnat(b, tiles, rep, lo=0, hi=S // 128):
                vT = tiles[2]
                for i in range(lo, hi):
                    pst = psA.tile([128, 128], bf16, tag="mm512",
                                   name=f"pst{rep}_{b}_{i}")
                    nc.tensor.transpose(
                        pst[:], vT[:, i * 128:(i + 1) * 128], id_sb[:]
                    )
                    nc.vector.tensor_copy(
                        ap3(vns[i][:, 0:1], 65, 2, 64),
                        ap3(pst[:, 0:1], 64, 2, 64))

            def scores(b, qb, tiles, att, rep, fill=()):
                """One 512-wide q block: per k-chunk, 2 concurrent head
                matmuls into a 2-bank PSUM tile + one merged-head exp."""
                qT, kT, vT = tiles
                q0 = qb * TB
                nkc = (q0 + TB) // 128
                fill = list(fill)
                nf = len(fill)
                fired = 0
                for kc in range(nkc):
                    kst = kc * 128
                    r0 = max(0, kst - q0)
                    n = TB - r0
                    ps = psS.tile([128, 1024], f32, tag="s",
                                  name=f"pss{rep}_{b}_{qb}_{kc}")
                    for h in range(2):
                        hs = slice(h * 64, (h + 1) * 64)
                        nc.tensor.matmul(
                            ps[:, h * 512:h * 512 + n],
                            kT[hs, kst:kst + 128],
                            qT[hs, q0 + r0:q0 + TB],
                            start=True, stop=True,
                            tile_position=(h * 64, 0),
                        )
                    base = att[:, (2 * kc) * TB + r0:(2 * kc) * TB + r0 + 1]
                    nc.scalar.activation(
                        ap3(base, TB, 2, n), ap3(ps[:, 0:1], 512, 2, n),
                        FT.Exp, scale=0.125,
                    )
                    if kst >= q0:
                        for h in range(2):
                            blk = att[:, (2 * kc + h) * TB + r0:
                                      (2 * kc + h) * TB + r0 + 128]
                            nc.vector.tensor_tensor(
                                blk, blk, tri_sb[:], op=MULT)
                    want = (kc + 1) * nf // nkc
                    while fired < want:
                        fill[fired]()
                        fired += 1
                while fired < nf:
                    fill[fired]()
                    fired += 1

            def attv(b, qb, h, att, aos, rep):
                q0 = qb * TB
                nkc = (q0 + TB) // 128
                ps_o = psO.tile([65, 512], f32, tag="o",
                                name=f"pso{rep}_{b}_{qb}_{h}")
                for kc in range(nkc):
                    lo = max(0, kc * 128 - q0)
                    nc.tensor.matmul(
                        ps_o[:, lo:TB],
                        vns[kc][:, h * 65:(h + 1) * 65],
                        att[:, (2 * kc + h) * TB + lo:(2 * kc + h + 1) * TB],
                        start=(kc == 0), stop=(kc == nkc - 1),
                    )
                rec = ms.tile([1, 512], f32, tag=f"rec{h}",
                              name=f"rec{rep}_{b}_{qb}_{h}")
                nc.vector.reciprocal(rec[:], ps_o[64:65, :])
                bc = ms.tile([64, 512], f32, tag=f"bc{h}",
                             name=f"bc{rep}_{b}_{qb}_{h}")
                nc.gpsimd.partition_broadcast(bc[:], rec[:])
                nc.vector.tensor_tensor(
                    aos[h * 64:(h + 1) * 64, q0:q0 + TB],
                    ps_o[0:64, :], bc[:], op=MULT,
                )

            def outproj_ec(b, half, ec, aos, rep):
                po = op.tile([128, 2 * TB], bf16, tag="po",
                             name=f"po{rep}_{b}_{ec}_{half}")
                for tt in range(2):
                    ps_p = psA.tile([128, TB], f32, tag="mm512",
                                    name=f"psp{rep}_{b}_{ec}_{half}_{tt}")
                    nc.tensor.matmul(
                        ps_p[:],
                        wo_sb[:, ec * 128:(ec + 1) * 128],
                        aos[:, half * 2 * TB + tt * TB:
                            half * 2 * TB + (tt + 1) * TB],
                        start=True, stop=True,
                    )
                    # ScalarE only when not competing with the exp FIFO
                    if (ec * 2 + tt) % 4 == 3:
                        nc.scalar.copy(po[:, tt * TB:(tt + 1) * TB], ps_p[:])
                    else:
                        nc.vector.tensor_copy(
                            po[:, tt * TB:(tt + 1) * TB], ps_p[:])
                nc.sync.dma_start(
                    outp.ap()[ec * 128:(ec + 1) * 128,
                              b * S + half * 2 * TB:
                              b * S + (half + 1) * 2 * TB],
                    po[:],
                )

            for rep in range(repeats):
                # prologue: batch-0 tokens [0,512) serially; the rest is
                # fill work inside the early scores loops
                tiles = alloc_qkv(0)
                xt = xt00 if rep == 0 else qkv_dma(0, 0, rep)
                for gi in range(3):
                    qkv_group(0, 0, tiles, xt, gi, rep)
                vnat(0, tiles, rep, 0, 4)
                pro_fill = []
                for t in (1, 2, 3):
                    xt = qkv_dma(0, t, rep)
                    for gi in range(3):
                        pro_fill.append(
                            (lambda t=t, x=xt, gi=gi, tl=tiles:
                             qkv_group(0, t, tl, x, gi, rep))
                        )
                    pro_fill.append(
                        (lambda t=t, tl=tiles: vnat(0, tl, rep, 4 * t, 4 * t + 4))
                    )
                prev = None  # (b, aos) with half-1 outproj still pending
                for b in range(B):
                    nxt = b + 1 if b + 1 < B else None
                    tiles_next = alloc_qkv(nxt) if nxt is not None else None
                    aos = aop.tile([128, S], bf16, tag="ao",
                                   name=f"ao{rep}_{b}")
                    for qb in range(NQ):
                        att = atp.tile([128, 32 * TB], bf16, tag="att",
                                       name=f"att{rep}_{b}_{qb}")
                        fill = []
                        if b == 0 and qb == 0:
                            fill.extend(pro_fill)
                        if nxt is not None and qb >= 1:
                            # next batch's QKV spread over qb 1..3
                            ts = [qb - 1] if qb < 3 else [2, 3]
                            for t in ts:
                                x1 = qkv_dma(nxt, t, rep)
                                for gi in range(3):
                                    fill.append(
                                        (lambda t=t, x=x1, gi=gi:
                                         qkv_group(nxt, t, tiles_next,
                                                   x, gi, rep))
                                    )
                        if qb == 0 and prev is not None:
                            pb, paos = prev
                            for ec in range(KC):
                                fill.append(
                                    (lambda ec=ec, pb=pb, paos=paos:
                                     outproj_ec(pb, 1, ec, paos, rep))
                                )
                            prev = None
                        if qb == 2:
                            for ec in range(KC):
                                fill.append(
                                    (lambda ec=ec: outproj_ec(b, 0, ec,
                                                              aos, rep))
                                )
                        scores(b, qb, tiles, att, rep, fill)
                        for h in range(2):
                            attv(b, qb, h, att, aos, rep)
                    if nxt is not None:
                        vnat(nxt, tiles_next, rep)
                    prev = (b, aos)
                    tiles = tiles_next
                pb, paos = prev
                for ec in range(KC):
                    outproj_ec(pb, 1, ec, paos, rep)
    nc.compile()
    return nc


_CACHE = {}


def _get_nc(repeats=1):
    if repeats not in _CACHE:
        _CACHE[repeats] = build(repeats)
    return _CACHE[repeats]


def make_in_maps(x, W_qkv, b_qkv, W_out, b_out):
    x = np.asarray(x, dtype=np.float32)
    W_qkv = np.asarray(W_qkv, dtype=np.float32)
    b_qkv = np.asarray(b_qkv, dtype=np.float32)
    W_out = np.asarray(W_out, dtype=np.float32)
    xT = np.ascontiguousarray(x.reshape(TOK, E).T).astype(BF)
    trim = np.ascontiguousarray(
        np.triu(np.ones((128, 128), dtype=np.float32))).astype(BF)
    ident = np.eye(128, dtype=np.float32).astype(BF)
    in_maps = []
    for c in range(8):
        cs = slice(c * 128, (c + 1) * 128)
        in_maps.append({
            "xT": xT,
            "wq": np.ascontiguousarray(
                W_qkv[:, c * 128:(c + 1) * 128]).astype(BF),
            "wk": np.ascontiguousarray(
                W_qkv[:, E + c * 128:E + (c + 1) * 128]).astype(BF),
            "wv": np.ascontiguousarray(
                W_qkv[:, 2 * E + c * 128:2 * E + (c + 1) * 128]).astype(BF),
            "wo": np.ascontiguousarray(W_out[cs, :]).astype(BF),
            "bq": np.ascontiguousarray(b_qkv[c * 128:(c + 1) * 128, None]),
            "bk": np.ascontiguousarray(
                b_qkv[E + c * 128:E + (c + 1) * 128, None]),
            "bv": np.ascontiguousarray(
                b_qkv[2 * E + c * 128:2 * E + (c + 1) * 128, None]),
            "tri": trim,
            "idd": ident,
        })
    return in_maps


def gather(results, b_out):
    total = np.zeros((E, TOK), dtype=np.float64)
    for c in range(8):
        total += results[c]["outp"].astype(np.float64)
    out = total.T.astype(np.float32) + np.asarray(b_out, dtype=np.float32)
    return np.ascontiguousarray(out.reshape(B, S, E)).astype(np.float32)


def kernel(x, W_qkv, b_qkv, W_out, b_out):
    nc = _get_nc(1)
    in_maps = make_in_maps(x, W_qkv, b_qkv, W_out, b_out)
    res = bass_utils.run_bass_kernel_spmd(nc, in_maps, core_ids=list(range(8)))
    return gather(res.results, b_out)
